# revision 1
# baseline (speedup 1.0000x reference)
"""DiagSSMBlock Trainium2 kernel.

Math (matches the reference exactly):
    s = b_mat.T @ x_seq.T                  # (H, T)
    y[h, t] = a[h] * y[h, t-1] + s[h, t]   # first-order IIR scan along t
    out = y.T                              # (T, H)

Sharding: a 2 (H) x 4 (T) grid over 8 cores. Each core computes a
(1024 channels x 1024 timesteps) output block: a (2048x1024)^T @
(2048x1024) matmul accumulated over K=2048 in PSUM, then the
per-channel IIR scan via the Vector engine's tensor_tensor_scan.

Time-sharding needs no cross-core communication: |a| <= sqrt(2/2048)
~ 0.031, so the scan state decays below fp32 noise within a few steps.
Each core's scan is seeded with a carry computed on the host from a
16-column warm-up strip (a^17 ~ 1e-25 of history is dropped -- exactly
zero in fp32). The strip matmul is 0.1% of the device FLOPs.

x is fed pre-transposed (K-major) from the host so both matmul operands
have the contraction dim in partitions; no on-chip transpose needed.
The matmul runs in float32r (full-rate relaxed fp32, ~1.5e-4 rel err);
set MM_DTYPE to float32 for the exact (4x slower) variant.
"""

import sys

import numpy as np

_REPO = "/opt/trn_rl_repo"
if _REPO not in sys.path:
    sys.path.insert(0, _REPO)

import concourse.bass as bass
import concourse.mybir as mybir
from concourse import bacc
from concourse.bass_utils import run_bass_kernel_spmd
from concourse.tile import TileContext

T = 4096
H = 2048
NCORES = 8
HG = 2           # h groups
TG = 4           # t groups
HSH = H // HG    # 1024 channels per core
TSH = T // TG    # 1024 timesteps per core
WARM = 16        # host-side scan warm-up columns per t boundary
P = 128
KT = H // P      # 16 k-tiles
MT = HSH // P    # 8 m-tiles
NCH = 512
CHUNKS = ((0, NCH), (NCH, NCH))  # matmul/scan t-chunks per core

MM_DTYPE = mybir.dt.float32r

_nc_cache = {}


def build_nc(mm_dtype=MM_DTYPE):
    f32 = mybir.dt.float32
    nc = bacc.Bacc(None, target_bir_lowering=False)

    xb = nc.declare_dram_parameter("xb", [H, TSH + HSH], mm_dtype, isOutput=False)
    av = nc.declare_dram_parameter("av", [HSH], f32, isOutput=False)
    cv = nc.declare_dram_parameter("cv", [HSH], f32, isOutput=False)
    y = nc.declare_dram_parameter("y", [HSH, TSH], f32, isOutput=True)

    xb_r = xb.rearrange("(ko p) t -> p ko t", p=P)  # [128, 16, 2048]: x cols then b cols
    av_r = av.rearrange("(mo p) -> p mo", p=P)      # [128, 8]
    cv_r = cv.rearrange("(mo p) -> p mo", p=P)      # [128, 8]
    y_r = y.rearrange("(mo p) t -> p mo t", p=P)    # [128, 8, 1024]

    NPAIR = MT // 2
    with TileContext(nc) as tc:
        with (
            tc.tile_pool(name="const", bufs=1) as cpool,
            tc.tile_pool(name="xp", bufs=KT) as xpool,
            tc.tile_pool(name="bp", bufs=KT * (NPAIR - 1)) as bpool,
            tc.tile_pool(name="yp", bufs=MT - 2) as ypool,
            tc.tile_pool(name="ypl", bufs=2) as ylpool,
            tc.tile_pool(name="ps0", bufs=4, space="PSUM") as p0pool,
            tc.tile_pool(name="ps1", bufs=4, space="PSUM") as p1pool,
        ):
            a_sb = cpool.tile([P, MT], f32)
            c_sb = cpool.tile([P, MT], f32)

            # One combined (x | pair-0 b) transfer per k-tile: the chase
            # stream lands under a single DMA completion per k, so the
            # first pair's matmuls wait on one semaphore instead of two.
            x_tiles = []
            b_tiles = {}
            for k in range(KT):
                xk = xpool.tile([P, TSH + 2 * P], mm_dtype, tag="x")
                nc.sync.dma_start(out=xk[:], in_=xb_r[:, k, 0 : TSH + 2 * P])
                x_tiles.append(xk)
                b_tiles[(k, 0)] = xk[:, TSH : TSH + 2 * P]
                if k == 0:
                    # a/c are only needed by the first scan, well after
                    # the stream-critical first x/b tiles.
                    nc.sync.dma_start(out=a_sb[:], in_=av_r[:])
                    nc.sync.dma_start(out=c_sb[:], in_=cv_r[:])
            for mp in range(1, NPAIR):
                for k in range(KT):
                    bk = bpool.tile([P, 2 * P], mm_dtype, tag="b")
                    nc.sync.dma_start(
                        out=bk[:],
                        in_=xb_r[:, k, TSH + mp * 2 * P : TSH + (mp + 1) * 2 * P],
                    )
                    b_tiles[(k, mp)] = bk

            for mp in range(NPAIR):
                pss = []
                for m2 in range(2):
                    p0 = p0pool.tile([P, NCH], f32, tag="ps0")
                    p1 = p1pool.tile([P, NCH], f32, tag="ps1")
                    pss.append((p0, p1))
                if mp < NPAIR - 1:
                    # k-major: chases the initial x/b DMA stream
                    for k in range(KT):
                        for m2 in range(2):
                            lhsT = b_tiles[(k, mp)][:, m2 * P : (m2 + 1) * P]
                            for ci, (c0, cw) in enumerate(CHUNKS):
                                nc.tensor.matmul(
                                    pss[m2][ci][:],
                                    lhsT,
                                    x_tiles[k][:, c0 : c0 + cw],
                                    start=(k == 0),
                                    stop=(k == KT - 1),
                                )
                else:
                    # Last pair: m-major and chunk-major, so every scan
                    # except the very last overlaps remaining matmuls,
                    # shrinking the kernel tail.
                    for m2 in range(2):
                        for ci, (c0, cw) in enumerate(CHUNKS):
                            for k in range(KT):
                                lhsT = b_tiles[(k, mp)][:, m2 * P : (m2 + 1) * P]
                                nc.tensor.matmul(
                                    pss[m2][ci][:],
                                    lhsT,
                                    x_tiles[k][:, c0 : c0 + cw],
                                    start=(k == 0),
                                    stop=(k == KT - 1),
                                )
                        m = 2 * mp + m2
                        ym = ylpool.tile([P, TSH], f32, tag="ylast")
                        for ci, (c0, cw) in enumerate(CHUNKS):
                            nc.vector.tensor_tensor_scan(
                                out=ym[:, c0 : c0 + cw],
                                data0=a_sb[:, m : m + 1].broadcast_to((P, cw)),
                                data1=pss[m2][ci][:],
                                initial=(
                                    c_sb[:, m : m + 1]
                                    if ci == 0
                                    else ym[:, c0 - 1 : c0]
                                ),
                                op0=mybir.AluOpType.mult,
                                op1=mybir.AluOpType.add,
                            )
                        nc.scalar.dma_start(out=y_r[:, m, :], in_=ym[:])
                if mp < NPAIR - 1:
                    for m2 in range(2):
                        m = 2 * mp + m2
                        ym = ypool.tile([P, TSH], f32, tag="y")
                        for ci, (c0, cw) in enumerate(CHUNKS):
                            nc.vector.tensor_tensor_scan(
                                out=ym[:, c0 : c0 + cw],
                                data0=a_sb[:, m : m + 1].broadcast_to((P, cw)),
                                data1=pss[m2][ci][:],
                                initial=(
                                    c_sb[:, m : m + 1]
                                    if ci == 0
                                    else ym[:, c0 - 1 : c0]
                                ),
                                op0=mybir.AluOpType.mult,
                                op1=mybir.AluOpType.add,
                            )
                        nc.scalar.dma_start(out=y_r[:, m, :], in_=ym[:])
    nc.finalize()
    return nc


def make_in_maps(x_seq, a_diag, b_mat):
    x_seq = np.ascontiguousarray(np.asarray(x_seq, dtype=np.float32))
    a_diag = np.ascontiguousarray(np.asarray(a_diag, dtype=np.float32))
    b_mat = np.ascontiguousarray(np.asarray(b_mat, dtype=np.float32))
    assert x_seq.shape == (T, H) and a_diag.shape == (H,) and b_mat.shape == (H, H)

    xT = np.ascontiguousarray(x_seq.T)  # (H, T), K-major for the PE

    # Scan warm-up carries at each t-block boundary: scan a 16-column
    # strip of s = b^T x from zero state. History older than the strip
    # contributes < |a|^17 ~ 1e-25 relative -- exactly zero in fp32.
    carries = np.zeros((TG, H), dtype=np.float32)
    for tg in range(1, TG):
        strip = b_mat.T @ xT[:, tg * TSH - WARM : tg * TSH]  # (H, WARM)
        state = np.zeros(H, dtype=np.float32)
        for j in range(WARM):
            state = a_diag * state + strip[:, j]
        carries[tg] = state

    in_maps = []
    for c in range(NCORES):
        hg, tg = divmod(c, TG)
        hsl = slice(hg * HSH, (hg + 1) * HSH)
        xb = np.concatenate(
            [xT[:, tg * TSH : (tg + 1) * TSH], b_mat[:, hsl]], axis=1
        )
        in_maps.append(
            {
                "xb": np.ascontiguousarray(xb),
                "av": np.ascontiguousarray(a_diag[hsl]),
                "cv": np.ascontiguousarray(carries[tg, hsl]),
            }
        )
    return in_maps


def run(in_maps, **kwargs):
    key = MM_DTYPE
    if key not in _nc_cache:
        _nc_cache[key] = build_nc(key)
    return run_bass_kernel_spmd(_nc_cache[key], in_maps, list(range(NCORES)), **kwargs)


def kernel(x_seq, a_diag, b_mat):
    res = run(make_in_maps(x_seq, a_diag, b_mat))
    yT = np.empty((H, T), dtype=np.float32)
    for c in range(NCORES):
        hg, tg = divmod(c, TG)
        yT[hg * HSH : (hg + 1) * HSH, tg * TSH : (tg + 1) * TSH] = res.results[c]["y"]
    return np.ascontiguousarray(yT.T)



# revision 4
# speedup vs baseline: 1.2060x; 1.2060x over previous
"""DiagSSMBlock Trainium2 kernel.

Math (matches the reference exactly):
    s = b_mat.T @ x_seq.T                  # (H, T)
    y[h, t] = a[h] * y[h, t-1] + s[h, t]   # first-order IIR scan along t
    out = y.T                              # (T, H)

Sharding: a 2 (H) x 4 (T) grid over 8 cores. Each core computes a
(1024 channels x 1024 timesteps) output block: a (2048x1024)^T @
(2048x1024) matmul accumulated over K=2048 in PSUM, then the
per-channel IIR scan via the Vector engine's tensor_tensor_scan.

Time-sharding needs no cross-core communication: |a| <= sqrt(2/2048)
~ 0.031, so the scan state decays below fp32 noise within a few steps.
Each core's scan is seeded with a carry computed on the host from a
16-column warm-up strip (a^17 ~ 1e-25 of history is dropped -- exactly
zero in fp32). The strip matmul is 0.1% of the device FLOPs.

x is fed pre-transposed (K-major) from the host so both matmul operands
have the contraction dim in partitions; no on-chip transpose needed.
The matmul runs in float32r (full-rate relaxed fp32, ~1.5e-4 rel err);
set MM_DTYPE to float32 for the exact (4x slower) variant.
"""

import sys

import ml_dtypes
import numpy as np

_REPO = "/opt/trn_rl_repo"
if _REPO not in sys.path:
    sys.path.insert(0, _REPO)

import concourse.bass as bass
import concourse.mybir as mybir
from concourse import bacc
from concourse.bass_utils import run_bass_kernel_spmd
from concourse.tile import TileContext

T = 4096
H = 2048
NCORES = 8
HG = 2           # h groups
TG = 4           # t groups
HSH = H // HG    # 1024 channels per core
TSH = T // TG    # 1024 timesteps per core
WARM = 16        # host-side scan warm-up columns per t boundary
P = 128
KT = H // P      # 16 k-tiles
MT = HSH // P    # 8 m-tiles
NCH = 512
CHUNKS = ((0, NCH), (NCH, NCH))  # matmul/scan t-chunks per core

MM_DTYPE = mybir.dt.bfloat16

_nc_cache = {}


def build_nc(mm_dtype=MM_DTYPE):
    f32 = mybir.dt.float32
    nc = bacc.Bacc(None, target_bir_lowering=False)

    xb = nc.declare_dram_parameter("xb", [H, TSH + HSH], mm_dtype, isOutput=False)
    av = nc.declare_dram_parameter("av", [HSH], f32, isOutput=False)
    cv = nc.declare_dram_parameter("cv", [HSH], f32, isOutput=False)
    y = nc.declare_dram_parameter("y", [HSH, TSH], f32, isOutput=True)

    xb_r = xb.rearrange("(ko p) t -> p ko t", p=P)  # [128, 16, 2048]: x cols then b cols
    av_r = av.rearrange("(mo p) -> p mo", p=P)      # [128, 8]
    cv_r = cv.rearrange("(mo p) -> p mo", p=P)      # [128, 8]
    y_r = y.rearrange("(mo p) t -> p mo t", p=P)    # [128, 8, 1024]

    NPAIR = MT // 2
    with TileContext(nc) as tc:
        with (
            tc.tile_pool(name="const", bufs=1) as cpool,
            tc.tile_pool(name="xp", bufs=KT) as xpool,
            tc.tile_pool(name="bp", bufs=KT * (NPAIR - 1)) as bpool,
            tc.tile_pool(name="yp", bufs=MT - 2) as ypool,
            tc.tile_pool(name="ypl", bufs=2) as ylpool,
            tc.tile_pool(name="ps0", bufs=4, space="PSUM") as p0pool,
            tc.tile_pool(name="ps1", bufs=4, space="PSUM") as p1pool,
        ):
            a_sb = cpool.tile([P, MT], f32)
            c_sb = cpool.tile([P, MT], f32)

            # One combined (x | pair-0 b) transfer per k-tile: the chase
            # stream lands under a single DMA completion per k, so the
            # first pair's matmuls wait on one semaphore instead of two.
            x_tiles = []
            b_tiles = {}
            for k in range(KT):
                xk = xpool.tile([P, TSH + 2 * P], mm_dtype, tag="x")
                nc.sync.dma_start(out=xk[:], in_=xb_r[:, k, 0 : TSH + 2 * P])
                x_tiles.append(xk)
                b_tiles[(k, 0)] = xk[:, TSH : TSH + 2 * P]
                if k == 0:
                    # a/c are only needed by the first scan, well after
                    # the stream-critical first x/b tiles.
                    nc.sync.dma_start(out=a_sb[:], in_=av_r[:])
                    nc.sync.dma_start(out=c_sb[:], in_=cv_r[:])
            for mp in range(1, NPAIR):
                for k in range(KT):
                    bk = bpool.tile([P, 2 * P], mm_dtype, tag="b")
                    nc.sync.dma_start(
                        out=bk[:],
                        in_=xb_r[:, k, TSH + mp * 2 * P : TSH + (mp + 1) * 2 * P],
                    )
                    b_tiles[(k, mp)] = bk

            for mp in range(NPAIR):
                pss = []
                for m2 in range(2):
                    p0 = p0pool.tile([P, NCH], f32, tag="ps0")
                    p1 = p1pool.tile([P, NCH], f32, tag="ps1")
                    pss.append((p0, p1))
                if mp < NPAIR - 1:
                    # k-major: chases the initial x/b DMA stream
                    for k in range(KT):
                        for m2 in range(2):
                            lhsT = b_tiles[(k, mp)][:, m2 * P : (m2 + 1) * P]
                            for ci, (c0, cw) in enumerate(CHUNKS):
                                nc.tensor.matmul(
                                    pss[m2][ci][:],
                                    lhsT,
                                    x_tiles[k][:, c0 : c0 + cw],
                                    start=(k == 0),
                                    stop=(k == KT - 1),
                                )
                else:
                    # Last pair: m-major and chunk-major, so every scan
                    # except the very last overlaps remaining matmuls,
                    # shrinking the kernel tail.
                    for m2 in range(2):
                        for ci, (c0, cw) in enumerate(CHUNKS):
                            for k in range(KT):
                                lhsT = b_tiles[(k, mp)][:, m2 * P : (m2 + 1) * P]
                                nc.tensor.matmul(
                                    pss[m2][ci][:],
                                    lhsT,
                                    x_tiles[k][:, c0 : c0 + cw],
                                    start=(k == 0),
                                    stop=(k == KT - 1),
                                )
                        m = 2 * mp + m2
                        ym = ylpool.tile([P, TSH], f32, tag="ylast")
                        for ci, (c0, cw) in enumerate(CHUNKS):
                            nc.vector.tensor_tensor_scan(
                                out=ym[:, c0 : c0 + cw],
                                data0=a_sb[:, m : m + 1].broadcast_to((P, cw)),
                                data1=pss[m2][ci][:],
                                initial=(
                                    c_sb[:, m : m + 1]
                                    if ci == 0
                                    else ym[:, c0 - 1 : c0]
                                ),
                                op0=mybir.AluOpType.mult,
                                op1=mybir.AluOpType.add,
                            )
                        nc.scalar.dma_start(out=y_r[:, m, :], in_=ym[:])
                if mp < NPAIR - 1:
                    for m2 in range(2):
                        m = 2 * mp + m2
                        ym = ypool.tile([P, TSH], f32, tag="y")
                        for ci, (c0, cw) in enumerate(CHUNKS):
                            nc.vector.tensor_tensor_scan(
                                out=ym[:, c0 : c0 + cw],
                                data0=a_sb[:, m : m + 1].broadcast_to((P, cw)),
                                data1=pss[m2][ci][:],
                                initial=(
                                    c_sb[:, m : m + 1]
                                    if ci == 0
                                    else ym[:, c0 - 1 : c0]
                                ),
                                op0=mybir.AluOpType.mult,
                                op1=mybir.AluOpType.add,
                            )
                        nc.scalar.dma_start(out=y_r[:, m, :], in_=ym[:])
    nc.finalize()
    return nc


def make_in_maps(x_seq, a_diag, b_mat):
    x_seq = np.ascontiguousarray(np.asarray(x_seq, dtype=np.float32))
    a_diag = np.ascontiguousarray(np.asarray(a_diag, dtype=np.float32))
    b_mat = np.ascontiguousarray(np.asarray(b_mat, dtype=np.float32))
    assert x_seq.shape == (T, H) and a_diag.shape == (H,) and b_mat.shape == (H, H)

    xT = np.ascontiguousarray(x_seq.T)  # (H, T), K-major for the PE

    # Scan warm-up carries at each t-block boundary: scan a 16-column
    # strip of s = b^T x from zero state. History older than the strip
    # contributes < |a|^17 ~ 1e-25 relative -- exactly zero in fp32.
    carries = np.zeros((TG, H), dtype=np.float32)
    for tg in range(1, TG):
        strip = b_mat.T @ xT[:, tg * TSH - WARM : tg * TSH]  # (H, WARM)
        state = np.zeros(H, dtype=np.float32)
        for j in range(WARM):
            state = a_diag * state + strip[:, j]
        carries[tg] = state

    in_maps = []
    for c in range(NCORES):
        hg, tg = divmod(c, TG)
        hsl = slice(hg * HSH, (hg + 1) * HSH)
        xb = np.concatenate(
            [xT[:, tg * TSH : (tg + 1) * TSH], b_mat[:, hsl]], axis=1
        ).astype(ml_dtypes.bfloat16)
        in_maps.append(
            {
                "xb": np.ascontiguousarray(xb),
                "av": np.ascontiguousarray(a_diag[hsl]),
                "cv": np.ascontiguousarray(carries[tg, hsl]),
            }
        )
    return in_maps


def run(in_maps, **kwargs):
    key = MM_DTYPE
    if key not in _nc_cache:
        _nc_cache[key] = build_nc(key)
    return run_bass_kernel_spmd(_nc_cache[key], in_maps, list(range(NCORES)), **kwargs)


def kernel(x_seq, a_diag, b_mat):
    res = run(make_in_maps(x_seq, a_diag, b_mat))
    yT = np.empty((H, T), dtype=np.float32)
    for c in range(NCORES):
        hg, tg = divmod(c, TG)
        yT[hg * HSH : (hg + 1) * HSH, tg * TSH : (tg + 1) * TSH] = res.results[c]["y"]
    return np.ascontiguousarray(yT.T)



# revision 5
# speedup vs baseline: 1.2281x; 1.0183x over previous
"""DiagSSMBlock Trainium2 kernel.

Math (matches the reference exactly):
    s = b_mat.T @ x_seq.T                  # (H, T)
    y[h, t] = a[h] * y[h, t-1] + s[h, t]   # first-order IIR scan along t
    out = y.T                              # (T, H)

Sharding: a 2 (H) x 4 (T) grid over 8 cores. Each core computes a
(1024 channels x 1024 timesteps) output block: a (2048x1024)^T @
(2048x1024) matmul accumulated over K=2048 in PSUM, then the
per-channel IIR scan via the Vector engine's tensor_tensor_scan.

Time-sharding needs no cross-core communication: |a| <= sqrt(2/2048)
~ 0.031, so the scan state decays below fp32 noise within a few steps.
Each core's scan is seeded with a carry computed on the host from a
16-column warm-up strip (a^17 ~ 1e-25 of history is dropped -- exactly
zero in fp32). The strip matmul is 0.1% of the device FLOPs.

Matmul operands are bf16 (PE streams bf16 at the same 1 column/cycle
as float32r but with half the HBM traffic; rel err ~2e-3 vs the 2e-2
budget). Accumulation stays fp32 in PSUM.

DMA strategy: every transfer is [128 partitions x contiguous bytes]
from a host-pre-swizzled DRAM image, so each dma_start is 128 large
descriptors. HWDGE descriptor generation costs ~0.5-0.7us *serialized*
per dma_start on the issuing engine, so the input stream is only
~22 dma_starts on the Sync ring (16 x k-tiles + 3 b-pair blocks + 1
const + the split first tile) instead of 130 row-tile transfers.
Outputs go on the Scalar HWDGE ring.

The PE is warmed with a few matmuls on a memset tile while the first
x k-tile streams in, so the HAM clock gate (1.2 -> 2.4 GHz after
~3.4us of sustained activity) is released before the real matmuls
start. The final m-tile is processed in fine chunks (256-col scans,
3-piece output DMA) to shrink the kernel tail.
"""

import sys

import ml_dtypes
import numpy as np

_REPO = "/opt/trn_rl_repo"
if _REPO not in sys.path:
    sys.path.insert(0, _REPO)

import concourse.bass as bass
import concourse.mybir as mybir
from concourse import bacc
from concourse.bass_utils import run_bass_kernel_spmd
from concourse.tile import TileContext

T = 4096
H = 2048
NCORES = 8
HG = 2           # h groups
TG = 4           # t groups
HSH = H // HG    # 1024 channels per core
TSH = T // TG    # 1024 timesteps per core
WARM = 16        # host-side scan warm-up columns per t boundary
P = 128
KT = H // P      # 16 k-tiles
MT = HSH // P    # 8 m-tiles
NPAIR = MT // 2  # 4 m-tile pairs
XW = TSH + 2 * P  # 1280 cols per x k-tile (x columns + pair-0 b columns)
BW = KT * 2 * P   # 4096 cols per b-pair block
NCH = 512
CHUNKS = ((0, NCH), (NCH, NCH))  # matmul/scan t-chunks per core
WU = 3           # PE warm-up matmuls (N=512) before the real stream

MM_DTYPE = mybir.dt.bfloat16
NP_MM = ml_dtypes.bfloat16

_nc_cache = {}


def build_nc(mm_dtype=MM_DTYPE):
    f32 = mybir.dt.float32
    nc = bacc.Bacc(None, target_bir_lowering=False)

    xkd = nc.declare_dram_parameter("xk", [P, KT * XW], mm_dtype, isOutput=False)
    bpd = nc.declare_dram_parameter("bp", [P, (NPAIR - 1) * BW], mm_dtype, isOutput=False)
    acvd = nc.declare_dram_parameter("acv", [P, 2 * MT], f32, isOutput=False)
    y = nc.declare_dram_parameter("y", [HSH, TSH], f32, isOutput=True)

    y_r = y.rearrange("(mo p) t -> p mo t", p=P)    # [128, 8, 1024]

    with TileContext(nc) as tc:
        with (
            tc.tile_pool(name="const", bufs=1) as cpool,
            tc.tile_pool(name="xp", bufs=KT) as xpool,
            tc.tile_pool(name="bpp", bufs=NPAIR - 1) as bpool,
            tc.tile_pool(name="yp", bufs=MT - 2) as ypool,
            tc.tile_pool(name="ypl", bufs=2) as ylpool,
            tc.tile_pool(name="ps0", bufs=4, space="PSUM") as p0pool,
            tc.tile_pool(name="ps1", bufs=4, space="PSUM") as p1pool,
        ):
            acv_sb = cpool.tile([P, 2 * MT], f32)
            wt = cpool.tile([P, NCH], mm_dtype)
            # Warm-up operand; lands on the DVE queue at body start, well
            # before the first x tile arrives.
            nc.vector.memset(wt[:], 0.0)

            # Input stream: x k-tile 0 is split (b columns first, then two
            # x halves) so the first matmul can start ~1.3us earlier than a
            # single 320KB transfer would allow. Everything else is one
            # dma_start per k-tile / per b-pair block.
            x_tiles = []
            b_tiles = {}
            for k in range(KT):
                xk = xpool.tile([P, XW], mm_dtype, tag="x")
                if k == 0:
                    nc.sync.dma_start(
                        out=xk[:, TSH:XW], in_=xkd[:, TSH:XW]
                    )
                    nc.sync.dma_start(
                        out=xk[:, 0:NCH], in_=xkd[:, 0:NCH]
                    )
                    nc.sync.dma_start(
                        out=xk[:, NCH:TSH], in_=xkd[:, NCH:TSH]
                    )
                    nc.sync.dma_start(out=acv_sb[:], in_=acvd[:])
                else:
                    nc.sync.dma_start(out=xk[:], in_=xkd[:, k * XW : (k + 1) * XW])
                x_tiles.append(xk)
                b_tiles[(k, 0)] = xk[:, TSH : TSH + 2 * P]
            # b pair 1 split in 4 so its head arrives before pair-1 matmuls
            # start; pairs 2-3 are single 1MB blocks with plenty of slack.
            bt1 = bpool.tile([P, BW], mm_dtype, tag="b")
            for q in range(4):
                nc.sync.dma_start(
                    out=bt1[:, q * (BW // 4) : (q + 1) * (BW // 4)],
                    in_=bpd[:, q * (BW // 4) : (q + 1) * (BW // 4)],
                )
            for k in range(KT):
                b_tiles[(k, 1)] = bt1[:, k * 2 * P : (k + 1) * 2 * P]
            for mp in range(2, NPAIR):
                bt = bpool.tile([P, BW], mm_dtype, tag="b")
                nc.sync.dma_start(
                    out=bt[:], in_=bpd[:, (mp - 1) * BW : mp * BW]
                )
                for k in range(KT):
                    b_tiles[(k, mp)] = bt[:, k * 2 * P : (k + 1) * 2 * P]

            # PE warm-up: junk matmuls on the memset tile release the HAM
            # clock gate while the first x tile streams in.
            wps = p0pool.tile([P, NCH], f32, tag="ps0")
            for _ in range(WU):
                nc.tensor.matmul(wps[:], wt[:, 0:P], wt[:], start=True, stop=True)

            for mp in range(NPAIR):
                pss = []
                for m2 in range(2):
                    p0 = p0pool.tile([P, NCH], f32, tag="ps0")
                    p1 = p1pool.tile([P, NCH], f32, tag="ps1")
                    pss.append((p0, p1))
                if mp < NPAIR - 1:
                    # k-major: chases the x DMA stream. Chunk order for k=0
                    # matches the split tile-0 arrival (both m2 on chunk 0
                    # first, so the second x half can land late).
                    for k in range(KT):
                        if k == 0:
                            order = [(m2, ci) for ci in range(2) for m2 in range(2)]
                        else:
                            order = [(m2, ci) for m2 in range(2) for ci in range(2)]
                        for m2, ci in order:
                            c0, cw = CHUNKS[ci]
                            lhsT = b_tiles[(k, mp)][:, m2 * P : (m2 + 1) * P]
                            nc.tensor.matmul(
                                pss[m2][ci][:],
                                lhsT,
                                x_tiles[k][:, c0 : c0 + cw],
                                start=(k == 0),
                                stop=(k == KT - 1),
                            )
                    for m2 in range(2):
                        m = 2 * mp + m2
                        ym = ypool.tile([P, TSH], f32, tag="y")
                        for ci, (c0, cw) in enumerate(CHUNKS):
                            nc.vector.tensor_tensor_scan(
                                out=ym[:, c0 : c0 + cw],
                                data0=acv_sb[:, m : m + 1].broadcast_to((P, cw)),
                                data1=pss[m2][ci][:],
                                initial=(
                                    acv_sb[:, MT + m : MT + m + 1]
                                    if ci == 0
                                    else ym[:, c0 - 1 : c0]
                                ),
                                op0=mybir.AluOpType.mult,
                                op1=mybir.AluOpType.add,
                            )
                        nc.scalar.dma_start(out=y_r[:, m, :], in_=ym[:])
                else:
                    # Last pair: chunk-major so scans overlap the remaining
                    # matmuls; the final m-tile finishes in 256-col scan
                    # chunks + 3-piece output DMA to minimise the tail.
                    m2 = 0
                    m = 2 * mp
                    ym = ylpool.tile([P, TSH], f32, tag="ylast")
                    for ci, (c0, cw) in enumerate(CHUNKS):
                        for k in range(KT):
                            lhsT = b_tiles[(k, mp)][:, 0:P]
                            nc.tensor.matmul(
                                pss[0][ci][:],
                                lhsT,
                                x_tiles[k][:, c0 : c0 + cw],
                                start=(k == 0),
                                stop=(k == KT - 1),
                            )
                        nc.vector.tensor_tensor_scan(
                            out=ym[:, c0 : c0 + cw],
                            data0=acv_sb[:, m : m + 1].broadcast_to((P, cw)),
                            data1=pss[0][ci][:],
                            initial=(
                                acv_sb[:, MT + m : MT + m + 1]
                                if ci == 0
                                else ym[:, c0 - 1 : c0]
                            ),
                            op0=mybir.AluOpType.mult,
                            op1=mybir.AluOpType.add,
                        )
                    nc.scalar.dma_start(out=y_r[:, m, :], in_=ym[:])

                    m = 2 * mp + 1
                    ym = ylpool.tile([P, TSH], f32, tag="ylast")
                    pA, pB = pss[1]
                    # chunk 0: cols 0:512 -> pA; chunk 1: 512:768 -> pB[0:256];
                    # chunk 2: 768:1024 -> pB[256:512]
                    for k in range(KT):
                        nc.tensor.matmul(
                            pA[:],
                            b_tiles[(k, mp)][:, P : 2 * P],
                            x_tiles[k][:, 0:NCH],
                            start=(k == 0),
                            stop=(k == KT - 1),
                        )
                    for ci in range(2):
                        for k in range(KT):
                            nc.tensor.matmul(
                                pB[:, ci * 256 : (ci + 1) * 256],
                                b_tiles[(k, mp)][:, P : 2 * P],
                                x_tiles[k][:, NCH + ci * 256 : NCH + (ci + 1) * 256],
                                start=(k == 0),
                                stop=(k == KT - 1),
                            )
                        if ci == 0:
                            # scans for cols 0:512 run under the chunk-2 matmuls
                            nc.vector.tensor_tensor_scan(
                                out=ym[:, 0:256],
                                data0=acv_sb[:, m : m + 1].broadcast_to((P, 256)),
                                data1=pA[:, 0:256],
                                initial=acv_sb[:, MT + m : MT + m + 1],
                                op0=mybir.AluOpType.mult,
                                op1=mybir.AluOpType.add,
                            )
                            nc.vector.tensor_tensor_scan(
                                out=ym[:, 256:512],
                                data0=acv_sb[:, m : m + 1].broadcast_to((P, 256)),
                                data1=pA[:, 256:512],
                                initial=ym[:, 255:256],
                                op0=mybir.AluOpType.mult,
                                op1=mybir.AluOpType.add,
                            )
                            nc.scalar.dma_start(out=y_r[:, m, 0:512], in_=ym[:, 0:512])
                            nc.vector.tensor_tensor_scan(
                                out=ym[:, 512:768],
                                data0=acv_sb[:, m : m + 1].broadcast_to((P, 256)),
                                data1=pB[:, 0:256],
                                initial=ym[:, 511:512],
                                op0=mybir.AluOpType.mult,
                                op1=mybir.AluOpType.add,
                            )
                            nc.scalar.dma_start(
                                out=y_r[:, m, 512:768], in_=ym[:, 512:768]
                            )
                    nc.vector.tensor_tensor_scan(
                        out=ym[:, 768:1024],
                        data0=acv_sb[:, m : m + 1].broadcast_to((P, 256)),
                        data1=pB[:, 256:512],
                        initial=ym[:, 767:768],
                        op0=mybir.AluOpType.mult,
                        op1=mybir.AluOpType.add,
                    )
                    nc.scalar.dma_start(out=y_r[:, m, 768:1024], in_=ym[:, 768:1024])
    nc.finalize()
    return nc


def make_in_maps(x_seq, a_diag, b_mat):
    x_seq = np.ascontiguousarray(np.asarray(x_seq, dtype=np.float32))
    a_diag = np.ascontiguousarray(np.asarray(a_diag, dtype=np.float32))
    b_mat = np.ascontiguousarray(np.asarray(b_mat, dtype=np.float32))
    assert x_seq.shape == (T, H) and a_diag.shape == (H,) and b_mat.shape == (H, H)

    xT = np.ascontiguousarray(x_seq.T)  # (H, T), K-major for the PE

    # Scan warm-up carries at each t-block boundary: scan a 16-column
    # strip of s = b^T x from zero state. History older than the strip
    # contributes < |a|^17 ~ 1e-25 relative -- exactly zero in fp32.
    carries = np.zeros((TG, H), dtype=np.float32)
    for tg in range(1, TG):
        strip = b_mat.T @ xT[:, tg * TSH - WARM : tg * TSH]  # (H, WARM)
        state = np.zeros(H, dtype=np.float32)
        for j in range(WARM):
            state = a_diag * state + strip[:, j]
        carries[tg] = state

    in_maps = []
    for c in range(NCORES):
        hg, tg = divmod(c, TG)
        hsl = slice(hg * HSH, (hg + 1) * HSH)
        xpart = xT[:, tg * TSH : (tg + 1) * TSH].reshape(KT, P, TSH)
        bcore = b_mat[:, hsl].reshape(KT, P, NPAIR, 2 * P)  # [k, p, mp, 256]
        xk = np.concatenate([xpart, bcore[:, :, 0, :]], axis=2)  # [k, p, 1280]
        xk = np.ascontiguousarray(
            xk.transpose(1, 0, 2).reshape(P, KT * XW).astype(NP_MM)
        )
        bp = np.ascontiguousarray(
            bcore[:, :, 1:, :]
            .transpose(1, 2, 0, 3)
            .reshape(P, (NPAIR - 1) * BW)
            .astype(NP_MM)
        )
        a_sw = a_diag[hsl].reshape(MT, P).T            # [128, 8]
        c_sw = carries[tg, hsl].reshape(MT, P).T       # [128, 8]
        acv = np.ascontiguousarray(
            np.concatenate([a_sw, c_sw], axis=1).astype(np.float32)
        )
        in_maps.append({"xk": xk, "bp": bp, "acv": acv})
    return in_maps


def run(in_maps, **kwargs):
    key = MM_DTYPE
    if key not in _nc_cache:
        _nc_cache[key] = build_nc(key)
    return run_bass_kernel_spmd(_nc_cache[key], in_maps, list(range(NCORES)), **kwargs)


def kernel(x_seq, a_diag, b_mat):
    res = run(make_in_maps(x_seq, a_diag, b_mat))
    yT = np.empty((H, T), dtype=np.float32)
    for c in range(NCORES):
        hg, tg = divmod(c, TG)
        yT[hg * HSH : (hg + 1) * HSH, tg * TSH : (tg + 1) * TSH] = res.results[c]["y"]
    return np.ascontiguousarray(yT.T)


# revision 11
# speedup vs baseline: 1.3004x; 1.0589x over previous
"""DiagSSMBlock Trainium2 kernel.

Math (matches the reference exactly):
    s = b_mat.T @ x_seq.T                  # (H, T)
    y[h, t] = a[h] * y[h, t-1] + s[h, t]   # first-order IIR scan along t
    out = y.T                              # (T, H)

Sharding: a 2 (H) x 4 (T) grid over 8 cores. Each core computes a
(1024 channels x 1024 timesteps) output block: a (2048x1024)^T @
(2048x1024) matmul accumulated over K=2048 in PSUM, then the
per-channel IIR scan via the Vector engine's tensor_tensor_scan.

Time-sharding needs no cross-core communication: |a| <= sqrt(2/2048)
~ 0.031, so the scan state decays below fp32 noise within a few steps.
Each core's scan is seeded with a carry computed on the host from a
16-column warm-up strip (a^17 ~ 1e-25 of history is dropped -- exactly
zero in fp32). The strip matmul is 0.1% of the device FLOPs.

Matmul operands are bf16 (PE streams bf16 at the same 1 column/cycle
as float32r but with half the HBM traffic; rel err ~2e-3 vs the 2e-2
budget). Accumulation stays fp32 in PSUM.

Schedule: the 8 output m-tiles are processed in groups of (3, 3, 2).
Group 0 runs k-major, chasing the x DMA stream; its 20.7us of PE work
covers the ~17us the full x stream needs at ~358 GB/s, so the PE --
not the DMA -- paces the kernel from the first tile onward. Groups 1-2
run m-major on resident data. The final m-tile finishes in fine chunks
(256-col scans on separate PSUM tiles, 3-piece output DMA) to minimise
the kernel tail.

DMA strategy: every transfer is [128 partitions x contiguous bytes]
from a host-pre-swizzled DRAM image, so each dma_start is 128 large
descriptors. HWDGE descriptor generation costs ~0.6us *serialized* per
dma_start on the issuing engine, so inputs are ~21 dma_starts on the
Sync ring instead of 130 row-tile transfers. Outputs go on the Scalar
HWDGE ring. The PE is warmed with junk matmuls on a memset tile while
the first x k-tile streams in, releasing the HAM clock gate (1.2 ->
2.4 GHz after ~3.4us of sustained activity) before the real stream.
"""

import sys

import ml_dtypes
import numpy as np

_REPO = "/opt/trn_rl_repo"
if _REPO not in sys.path:
    sys.path.insert(0, _REPO)

import concourse.bass as bass
import concourse.mybir as mybir
from concourse import bacc
from concourse.bass_utils import run_bass_kernel_spmd
from concourse.tile import TileContext

T = 4096
H = 2048
NCORES = 8
HG = 2           # h groups
TG = 4           # t groups
HSH = H // HG    # 1024 channels per core
TSH = T // TG    # 1024 timesteps per core
WARM = 16        # host-side scan warm-up columns per t boundary
P = 128
KT = H // P      # 16 k-tiles
MT = HSH // P    # 8 m-tiles
GROUPS = ((0, 1, 2), (3, 4, 5), (6, 7))
G0W = len(GROUPS[0]) * P      # 384 b cols carried with each x k-tile
XW = TSH + G0W                # 1408 cols per x k-tile
B1W = KT * len(GROUPS[1]) * P  # group-1 b block cols
B2W = KT * len(GROUPS[2]) * P  # group-2 b block cols
NCH = 512
CHUNKS = ((0, NCH), (NCH, NCH))
WU = 3           # PE warm-up matmuls (N=512) before the real stream

MM_DTYPE = mybir.dt.bfloat16
NP_MM = ml_dtypes.bfloat16

_nc_cache = {}


def build_nc(mm_dtype=MM_DTYPE):
    f32 = mybir.dt.float32
    nc = bacc.Bacc(None, target_bir_lowering=False)

    xkd = nc.declare_dram_parameter("xk", [P, KT * XW], mm_dtype, isOutput=False)
    bpd = nc.declare_dram_parameter("bp", [P, B1W + B2W], mm_dtype, isOutput=False)
    acvd = nc.declare_dram_parameter("acv", [P, 2 * MT], f32, isOutput=False)
    y = nc.declare_dram_parameter("y", [HSH, TSH], f32, isOutput=True)

    y_r = y.rearrange("(mo p) t -> p mo t", p=P)    # [128, 8, 1024]

    with TileContext(nc) as tc:
        with (
            tc.tile_pool(name="const", bufs=1) as cpool,
            tc.tile_pool(name="xp", bufs=KT) as xpool,
            tc.tile_pool(name="bpp", bufs=2) as bpool,
            tc.tile_pool(name="yp", bufs=MT - 2) as ypool,
            tc.tile_pool(name="ypl", bufs=2) as ylpool,
            tc.tile_pool(name="ps0", bufs=4, space="PSUM") as p0pool,
            tc.tile_pool(name="ps1", bufs=4, space="PSUM") as p1pool,
        ):
            acv_sb = cpool.tile([P, 2 * MT], f32)
            wt = cpool.tile([P, NCH], mm_dtype)
            # Warm-up operand on the GpSimd queue -- free earliest.
            nc.gpsimd.memset(wt[:], 0.0)

            # Input stream: x k-tile 0 is split (b columns first, then two
            # x halves) so the first matmul starts ~1.3us earlier than one
            # 360KB transfer would allow.
            x_tiles = []
            for k in range(KT):
                xk = xpool.tile([P, XW], mm_dtype, tag="x")
                if k == 0:
                    nc.sync.dma_start(out=xk[:, TSH:XW], in_=xkd[:, TSH:XW])
                    nc.sync.dma_start(out=xk[:, 0:NCH], in_=xkd[:, 0:NCH])
                    nc.sync.dma_start(out=xk[:, NCH:TSH], in_=xkd[:, NCH:TSH])
                else:
                    nc.sync.dma_start(out=xk[:], in_=xkd[:, k * XW : (k + 1) * XW])
                if k == 3:
                    nc.sync.dma_start(out=acv_sb[:], in_=acvd[:])
                x_tiles.append(xk)
            bt1 = bpool.tile([P, B1W], mm_dtype, tag="b")
            nc.sync.dma_start(out=bt1[:], in_=bpd[:, 0:B1W])
            bt2 = bpool.tile([P, B2W], mm_dtype, tag="b")
            nc.sync.dma_start(out=bt2[:], in_=bpd[:, B1W : B1W + B2W])

            def b_tile(k, m):
                if m < 3:
                    return x_tiles[k][:, TSH + m * P : TSH + (m + 1) * P]
                if m < 6:
                    return bt1[:, k * G0W + (m - 3) * P : k * G0W + (m - 2) * P]
                return bt2[:, k * 2 * P + (m - 6) * P : k * 2 * P + (m - 5) * P]

            def scan(ym, m, c0, cw, data1, first):
                nc.vector.tensor_tensor_scan(
                    out=ym[:, c0 : c0 + cw],
                    data0=acv_sb[:, m : m + 1].broadcast_to((P, cw)),
                    data1=data1,
                    initial=(
                        acv_sb[:, MT + m : MT + m + 1]
                        if first
                        else ym[:, c0 - 1 : c0]
                    ),
                    op0=mybir.AluOpType.mult,
                    op1=mybir.AluOpType.add,
                )

            # PE warm-up: junk matmuls release the HAM clock gate while the
            # first x tile streams in.
            wps = p0pool.tile([P, NCH], f32, tag="ps0")
            for _ in range(WU):
                nc.tensor.matmul(wps[:], wt[:, 0:P], wt[:], start=True, stop=True)

            # ---- Group 0: k-major, chases the x DMA stream ----
            g0 = GROUPS[0]
            ps = {}
            for m in g0:
                ps[(m, 0)] = p0pool.tile([P, NCH], f32, tag="ps0", name=f"ps0_m{m}")
                ps[(m, 1)] = p1pool.tile([P, NCH], f32, tag="ps1", name=f"ps1_m{m}")
            for k in range(KT):
                if k == 0:
                    order = [(ci, m) for ci in range(2) for m in g0]
                else:
                    order = [(ci, m) for m in g0 for ci in range(2)]
                for ci, m in order:
                    c0, cw = CHUNKS[ci]
                    nc.tensor.matmul(
                        ps[(m, ci)][:],
                        b_tile(k, m),
                        x_tiles[k][:, c0 : c0 + cw],
                        start=(k == 0),
                        stop=(k == KT - 1),
                    )
            for m in g0:
                ym = ypool.tile([P, TSH], f32, tag="y")
                for ci, (c0, cw) in enumerate(CHUNKS):
                    scan(ym, m, c0, cw, ps[(m, ci)][:], ci == 0)
                nc.scalar.dma_start(out=y_r[:, m, :], in_=ym[:])

            # ---- Group 1: m-major on resident data ----
            for m in GROUPS[1]:
                pm = (
                    p0pool.tile([P, NCH], f32, tag="ps0", name=f"ps0_m{m}"),
                    p1pool.tile([P, NCH], f32, tag="ps1", name=f"ps1_m{m}"),
                )
                for k in range(KT):
                    for ci, (c0, cw) in enumerate(CHUNKS):
                        nc.tensor.matmul(
                            pm[ci][:],
                            b_tile(k, m),
                            x_tiles[k][:, c0 : c0 + cw],
                            start=(k == 0),
                            stop=(k == KT - 1),
                        )
                ym = ypool.tile([P, TSH], f32, tag="y")
                for ci, (c0, cw) in enumerate(CHUNKS):
                    scan(ym, m, c0, cw, pm[ci][:], ci == 0)
                nc.scalar.dma_start(out=y_r[:, m, :], in_=ym[:])

            # ---- Group 2: m6 chunk-major, m7 fine-grained tail ----
            m = 6
            pm = (
                p0pool.tile([P, NCH], f32, tag="ps0", name="ps0_m6"),
                p1pool.tile([P, NCH], f32, tag="ps1", name="ps1_m6"),
            )
            ym = ylpool.tile([P, TSH], f32, tag="ylast")
            for ci, (c0, cw) in enumerate(CHUNKS):
                for k in range(KT):
                    nc.tensor.matmul(
                        pm[ci][:],
                        b_tile(k, m),
                        x_tiles[k][:, c0 : c0 + cw],
                        start=(k == 0),
                        stop=(k == KT - 1),
                    )
                scan(ym, m, c0, cw, pm[ci][:], ci == 0)
            nc.scalar.dma_start(out=y_r[:, m, :], in_=ym[:])

            m = 7
            pA = p0pool.tile([P, NCH], f32, tag="ps0")
            pB1f = p1pool.tile([P, NCH], f32, tag="ps1")
            pB2f = p0pool.tile([P, NCH], f32, tag="ps0")
            pB1 = pB1f[:, 0:256]
            pB2 = pB2f[:, 0:256]
            ym = ylpool.tile([P, TSH], f32, tag="ylast")
            for k in range(KT):
                nc.tensor.matmul(
                    pA[:], b_tile(k, m), x_tiles[k][:, 0:NCH],
                    start=(k == 0), stop=(k == KT - 1),
                )
            for k in range(KT):
                nc.tensor.matmul(
                    pB1[:], b_tile(k, m), x_tiles[k][:, NCH : NCH + 256],
                    start=(k == 0), stop=(k == KT - 1),
                )
            scan(ym, m, 0, 256, pA[:, 0:256], True)
            scan(ym, m, 256, 256, pA[:, 256:512], False)
            nc.scalar.dma_start(out=y_r[:, m, 0:512], in_=ym[:, 0:512])
            for k in range(KT):
                nc.tensor.matmul(
                    pB2[:], b_tile(k, m), x_tiles[k][:, NCH + 256 : TSH],
                    start=(k == 0), stop=(k == KT - 1),
                )
            scan(ym, m, 512, 256, pB1[:], False)
            nc.scalar.dma_start(out=y_r[:, m, 512:768], in_=ym[:, 512:768])
            scan(ym, m, 768, 256, pB2[:], False)
            nc.scalar.dma_start(out=y_r[:, m, 768:1024], in_=ym[:, 768:1024])
    nc.finalize()
    return nc


def make_in_maps(x_seq, a_diag, b_mat):
    x_seq = np.ascontiguousarray(np.asarray(x_seq, dtype=np.float32))
    a_diag = np.ascontiguousarray(np.asarray(a_diag, dtype=np.float32))
    b_mat = np.ascontiguousarray(np.asarray(b_mat, dtype=np.float32))
    assert x_seq.shape == (T, H) and a_diag.shape == (H,) and b_mat.shape == (H, H)

    xT = np.ascontiguousarray(x_seq.T)  # (H, T), K-major for the PE

    # Scan warm-up carries at each t-block boundary: scan a 16-column
    # strip of s = b^T x from zero state. History older than the strip
    # contributes < |a|^17 ~ 1e-25 relative -- exactly zero in fp32.
    carries = np.zeros((TG, H), dtype=np.float32)
    for tg in range(1, TG):
        strip = b_mat.T @ xT[:, tg * TSH - WARM : tg * TSH]  # (H, WARM)
        state = np.zeros(H, dtype=np.float32)
        for j in range(WARM):
            state = a_diag * state + strip[:, j]
        carries[tg] = state

    in_maps = []
    for c in range(NCORES):
        hg, tg = divmod(c, TG)
        hsl = slice(hg * HSH, (hg + 1) * HSH)
        xpart = xT[:, tg * TSH : (tg + 1) * TSH].reshape(KT, P, TSH)
        bcore = b_mat[:, hsl].reshape(KT, P, HSH)       # [k, p, 1024]
        xk = np.concatenate([xpart, bcore[:, :, 0:G0W]], axis=2)  # [k, p, 1408]
        xk = np.ascontiguousarray(
            xk.transpose(1, 0, 2).reshape(P, KT * XW).astype(NP_MM)
        )
        b1 = bcore[:, :, G0W : 2 * G0W].transpose(1, 0, 2).reshape(P, B1W)
        b2 = bcore[:, :, 2 * G0W : HSH].transpose(1, 0, 2).reshape(P, B2W)
        bp = np.ascontiguousarray(
            np.concatenate([b1, b2], axis=1).astype(NP_MM)
        )
        a_sw = a_diag[hsl].reshape(MT, P).T            # [128, 8]
        c_sw = carries[tg, hsl].reshape(MT, P).T       # [128, 8]
        acv = np.ascontiguousarray(
            np.concatenate([a_sw, c_sw], axis=1).astype(np.float32)
        )
        in_maps.append({"xk": xk, "bp": bp, "acv": acv})
    return in_maps


def run(in_maps, **kwargs):
    key = MM_DTYPE
    if key not in _nc_cache:
        _nc_cache[key] = build_nc(key)
    return run_bass_kernel_spmd(_nc_cache[key], in_maps, list(range(NCORES)), **kwargs)


def kernel(x_seq, a_diag, b_mat):
    res = run(make_in_maps(x_seq, a_diag, b_mat))
    yT = np.empty((H, T), dtype=np.float32)
    for c in range(NCORES):
        hg, tg = divmod(c, TG)
        yT[hg * HSH : (hg + 1) * HSH, tg * TSH : (tg + 1) * TSH] = res.results[c]["y"]
    return np.ascontiguousarray(yT.T)
